# revision 6
# baseline (speedup 1.0000x reference)
"""Trainium2 Bass kernel for nn_BILINEAR_56169582297414 (gnn message passing).

Reference computation (per prediction pair b):
    item_e = item_table[item_inputs[b]]                    # [D]
    mem_e  = user_table[member_ids[b, :]]                  # [M, D]
    scores[m] = mem_e[m] @ W_bil @ item_e + b_bil          # bilinear
    w = scores * member_mask[b]                            # mask padded members
    fu = sum_m w[m] * mem_e[m]                             # [D]
    ne = [fu * item_e, fu, item_e]                         # [3D]
    y = sigmoid(relu(ne @ W1.T + b1) @ W2.T + b2)          # [1]

Strategy: data-parallel over 8 NeuronCores, tables replicated. The serial
resource is SWDGE gather descriptor supply (~11 ns/desc per queue pair,
~2.8 ns/desc aggregate over the 4 SWDGE queues). So:
  - rows sorted by true group length (desc) and striped across cores; each
    tile fetches only its max length (~1.9x fewer member rows);
  - member and item tables both bf16 4-packed (256B elements); gathers are
    assigned to the least-loaded of the 4 SWDGE queues (4 Q7 core pairs);
    item gathers batched 4 groups per instruction;
  - dynamic_dma_scratch_size doubled so each queue's descriptor ring holds
    ~2 gathers, reducing decode-stage await_space head-of-line blocking;
  - packed-row selects via copy_predicated only (DVE COPY opcode is ~4x
    slower) on uint32-bitcast views to halve element count;
  - the weighted member sum uses a pairwise add tree over contiguous views
    (the strided [g,d,m] TENSOR_REDUCE it replaces ran ~20x slower);
  - everything up to PSUM runs in bf16 (2x DVE and PE); PSUM accumulation
    and the score/activation nonlinearities stay fp32.
"""

import sys

sys.path.insert(0, "/opt/trn_rl_repo")

import numpy as np

B = 262144
M = 16
NU = 100000
NI = 50000
D = 32
N_CORES = 8
BC = B // N_CORES
P = 128
NT = BC // P
G = 4

_COMPILED = {}


def _group_gl(prof, g=G):
    """Per-group max member count (prof is non-increasing)."""
    return [max(prof[i * g : (i + 1) * g]) for i in range(len(prof) // g)]


def _next_pow2(x):
    h = 1
    while h < x:
        h *= 2
    return h


def _queue_plan(gls, g=G):
    """Greedy least-loaded queue for each gather, in emit order.

    Returns (member_q[gi], quad_q[qk]) lists."""
    load = [0, 0, 0, 0]
    member_q = [0] * len(gls)
    quad_q = [0] * (len(gls) // 4)
    for gi in range(len(gls)):
        if gi % 4 == 0:
            q = min(range(4), key=lambda i: (load[i], i))
            quad_q[gi // 4] = q
            load[q] += 16 * 128
        q = min(range(4), key=lambda i: (load[i], i))
        member_q[gi] = q
        load[q] += g * gls[gi] * 128
    return member_q, quad_q


def build_kernel(bc, g=G, prof=None):
    import concourse.bacc as bacc
    import concourse.tile as tile
    from concourse import mybir
    from concourse.library_config import mlp

    nt = bc // P
    assert nt % g == 0
    ngroups = nt // g
    assert ngroups % 4 == 0
    if prof is None:
        prof = [M] * nt
    prof = [int(max(1, min(M, x))) for x in prof]
    gls = _group_gl(prof, g)
    member_q, quad_q = _queue_plan(gls, g)
    dt = mybir.dt

    # flat col offsets for per-group idx / mask-blob tensors
    idx_cols = [g * gl * 8 for gl in gls]            # int16 cols ([128, .])
    blob_cols = [4 * g * gl + 4 * g for gl in gls]   # uint8: 4 msel + 4 isel
    idx_off = np.concatenate([[0], np.cumsum(idx_cols)]).astype(int)
    blob_off = np.concatenate([[0], np.cumsum(blob_cols)]).astype(int)

    nc = bacc.Bacc("TRN2", target_bir_lowering=False, debug=False,
                   num_swdge_queues=4, dynamic_dma_scratch_size=32768)

    ids16 = nc.dram_tensor("ids16", [P, int(idx_off[-1])], dt.int16,
                           kind="ExternalInput")
    iid16 = nc.dram_tensor("iid16", [P, (ngroups // 4) * 16 * 8], dt.int16,
                           kind="ExternalInput")
    mblob = nc.dram_tensor("mblob", [P, int(blob_off[-1])], dt.uint8,
                           kind="ExternalInput")
    mask = nc.dram_tensor("mask", [bc, M], dt.float32, kind="ExternalInput")
    user4 = nc.dram_tensor("user4", [NU // 4, 4 * D], dt.bfloat16,
                           kind="ExternalInput")
    item4 = nc.dram_tensor("item4", [NI // 4, 4 * D], dt.bfloat16,
                           kind="ExternalInput")
    w_bil_t = nc.dram_tensor("w_bil_t", [D, D], dt.bfloat16, kind="ExternalInput")
    w1_t = nc.dram_tensor("w1_t", [3 * D, 8], dt.bfloat16, kind="ExternalInput")
    w2_t = nc.dram_tensor("w2_t", [8, 1], dt.bfloat16, kind="ExternalInput")
    b1 = nc.dram_tensor("b1", [8, 1], dt.float32, kind="ExternalInput")
    b2 = nc.dram_tensor("b2", [1, 1], dt.float32, kind="ExternalInput")
    bbil = nc.dram_tensor("bbil", [P, 1], dt.float32, kind="ExternalInput")
    ident = nc.dram_tensor("ident", [P, P], dt.bfloat16, kind="ExternalInput")
    y_out = nc.dram_tensor("y", [nt, P], dt.float32, kind="ExternalOutput")

    GM = g * M
    GNE = g * 3 * D
    GP = g * P

    with tile.TileContext(nc) as tc:
        with (
            tc.tile_pool(name="const", bufs=1) as cpool,
            tc.tile_pool(name="io", bufs=6) as iopool,
            tc.tile_pool(name="work", bufs=3) as wpool,
            tc.tile_pool(name="gath", bufs=6) as gpool,
            tc.tile_pool(name="quad", bufs=2) as qpool,
            tc.tile_pool(name="prodp", bufs=2) as prpool,
            tc.tile_pool(name="psum", bufs=1, space="PSUM") as ppool,
            tc.tile_pool(name="psumv", bufs=2, space="PSUM") as ppoolv,
        ):
            with tc.tile_critical():
                nc.gpsimd.load_library(mlp)

            wt_sb = cpool.tile([D, D], dt.bfloat16, tag="wt")
            nc.sync.dma_start(out=wt_sb[:], in_=w_bil_t[:])
            w1_sb = cpool.tile([3 * D, 8], dt.bfloat16, tag="w1")
            nc.sync.dma_start(out=w1_sb[:], in_=w1_t[:])
            w2_sb = cpool.tile([8, 1], dt.bfloat16, tag="w2")
            nc.sync.dma_start(out=w2_sb[:], in_=w2_t[:])
            b1_sb = cpool.tile([8, 1], dt.float32, tag="b1")
            nc.sync.dma_start(out=b1_sb[:], in_=b1[:])
            b2_sb = cpool.tile([1, 1], dt.float32, tag="b2")
            nc.sync.dma_start(out=b2_sb[:], in_=b2[:])
            bbil_sb = cpool.tile([P, 1], dt.float32, tag="bbil")
            nc.sync.dma_start(out=bbil_sb[:], in_=bbil[:])
            id_sb = cpool.tile([P, P], dt.bfloat16, tag="ident")
            nc.sync.dma_start(out=id_sb[:], in_=ident[:])

            quad_sb = None
            for gi in range(ngroups):
                r0 = gi * g * P
                GL = gls[gi]
                Hp = _next_pow2(GL)
                nmi = g * GL * 128  # member idxs this group

                if gi % 4 == 0:
                    qk = gi // 4
                    iid_sb = iopool.tile([P, 16 * 8], dt.int16, tag="iid")
                    nc.sync.dma_start(
                        out=iid_sb[:],
                        in_=iid16[:, qk * 128 : (qk + 1) * 128],
                    )
                    quad_sb = qpool.tile([P, 16 * 4 * D], dt.bfloat16, tag="q4")
                    q4 = quad_sb[:].rearrange("p (c e) -> p c e", c=16)
                    nc.gpsimd.dma_gather(
                        out_ap=q4,
                        in_ap=item4[:],
                        idxs_ap=iid_sb[:],
                        num_idxs=16 * 128,
                        num_idxs_reg=16 * 128,
                        elem_size=4 * D,
                        single_packet=False,
                        queue_num=quad_q[qk],
                    )

                ids_sb = iopool.tile([P, idx_cols[gi]], dt.int16, tag="ids")
                nc.sync.dma_start(
                    out=ids_sb[:],
                    in_=ids16[:, int(idx_off[gi]) : int(idx_off[gi + 1])],
                )

                # Batched member gather: idx i=(j*GL+m)*128+p -> block j*GL+m,
                # partition p; each 256B element = 4 packed bf16 table rows.
                g4_sb = gpool.tile([P, g * GL * 4 * D], dt.bfloat16, tag="g4")
                g4 = g4_sb[:].rearrange("p (c e) -> p c e", c=g * GL)
                nc.gpsimd.dma_gather(
                    out_ap=g4,
                    in_ap=user4[:],
                    idxs_ap=ids_sb[:],
                    num_idxs=nmi,
                    num_idxs_reg=nmi,
                    elem_size=4 * D,
                    single_packet=False,
                    queue_num=member_q[gi],
                )

                blob_sb = iopool.tile([P, blob_cols[gi]], dt.uint8, tag="blob")
                nc.sync.dma_start(
                    out=blob_sb[:],
                    in_=mblob[:, int(blob_off[gi]) : int(blob_off[gi + 1])],
                )
                ms_sb = [
                    blob_sb[:, q * g * GL : (q + 1) * g * GL] for q in range(4)
                ]
                is_sb = [
                    blob_sb[:, 4 * g * GL + q * g : 4 * g * GL + (q + 1) * g]
                    for q in range(4)
                ]
                mask_sb = iopool.tile([P, GM], dt.float32, tag="mask")
                nc.sync.dma_start(
                    out=mask_sb[:].rearrange("p (g m) -> p g m", g=g),
                    in_=mask[r0 : r0 + g * P, :].rearrange("(g p) m -> p g m", p=P),
                )

                # 1-of-4 sub-row select on uint32 views -> mem [P,(g,GL),D] bf16
                mem_sb = wpool.tile([P, g * GL * D], dt.bfloat16, tag="mem")
                g4u = (
                    g4_sb[:]
                    .bitcast(dt.uint32)
                    .rearrange("p (c e) -> p c e", c=g * GL)
                )
                memu = (
                    mem_sb[:]
                    .bitcast(dt.uint32)
                    .rearrange("p (c e) -> p c e", c=g * GL)
                )
                HD = D // 2  # uint32 words per row
                for q in range(4):
                    nc.vector.copy_predicated(
                        out=memu,
                        mask=ms_sb[q].unsqueeze(2).broadcast_to([P, g * GL, HD]),
                        data=g4u[:, :, q * HD : (q + 1) * HD],
                    )

                ne_sb = wpool.tile([P, GNE], dt.bfloat16, tag="ne")
                ne3 = ne_sb[:].rearrange("p (g c) -> p g c", g=g)
                neu = (
                    ne_sb[:]
                    .bitcast(dt.uint32)
                    .rearrange("p (g c) -> p g c", g=g)
                )
                q4u = (
                    quad_sb[:]
                    .bitcast(dt.uint32)
                    .rearrange("p (c e) -> p c e", c=16)
                )
                qbase = (gi % 4) * g
                for q in range(4):
                    nc.vector.copy_predicated(
                        out=neu[:, :, D : D + HD],
                        mask=is_sb[q].unsqueeze(2).broadcast_to([P, g, HD]),
                        data=q4u[:, qbase : qbase + g, q * HD : (q + 1) * HD],
                    )

                itemT_ps = ppool.tile([D, GP], dt.bfloat16, tag="itemT", space="PSUM")
                for j in range(g):
                    nc.tensor.transpose(
                        out=itemT_ps[:, j * P : (j + 1) * P],
                        in_=ne3[:, j, 2 * D : 3 * D],
                        identity=id_sb[:],
                    )
                itemT_sb = wpool.tile([D, GP], dt.bfloat16, tag="itemT")
                nc.scalar.activation(
                    out=itemT_sb[:],
                    in_=itemT_ps[:],
                    func=mybir.ActivationFunctionType.Copy,
                )

                v_ps = ppoolv.tile([P, g * D], dt.float32, tag="v", space="PSUM")
                for j in range(g):
                    nc.tensor.matmul(
                        v_ps[:, j * D : (j + 1) * D],
                        lhsT=itemT_sb[:, j * P : (j + 1) * P],
                        rhs=wt_sb[:],
                        start=True,
                        stop=True,
                    )
                v16_sb = wpool.tile([P, g * D], dt.bfloat16, tag="v16")
                nc.scalar.activation(
                    out=v16_sb[:],
                    in_=v_ps[:],
                    func=mybir.ActivationFunctionType.Copy,
                )

                mem4 = mem_sb[:].rearrange("p (g m d) -> p g m d", g=g, m=GL)
                v_b = (
                    v16_sb[:]
                    .rearrange("p (g d) -> p g d", g=g)
                    .unsqueeze(2)
                    .broadcast_to([P, g, GL, D])
                )
                prod_sb = prpool.tile([P, GM * D], dt.bfloat16, tag="prod")
                prod4 = prod_sb[:].rearrange("p (g m d) -> p g m d", g=g, m=M)[
                    :, :, :GL, :
                ]
                nc.vector.tensor_mul(out=prod4, in0=mem4, in1=v_b)

                scores_sb = wpool.tile([P, GM], dt.float32, tag="scores")
                sc3 = scores_sb[:].rearrange("p (g m) -> p g m", g=g)
                nc.vector.reduce_sum(
                    out=sc3[:, :, :GL], in_=prod4, axis=mybir.AxisListType.X
                )

                w16_sb = wpool.tile([P, GM], dt.bfloat16, tag="w16")
                w3 = w16_sb[:].rearrange("p (g m) -> p g m", g=g)
                m3 = mask_sb[:].rearrange("p (g m) -> p g m", g=g)
                nc.vector.scalar_tensor_tensor(
                    out=w3[:, :, :GL],
                    in0=sc3[:, :, :GL],
                    scalar=bbil_sb[:, :1],
                    in1=m3[:, :, :GL],
                    op0=mybir.AluOpType.add,
                    op1=mybir.AluOpType.mult,
                )

                # weighted members; zero-pad to next pow2 for the add tree
                wp_sb = prpool.tile([P, GM * D], dt.bfloat16, tag="wprod")
                wp4 = wp_sb[:].rearrange("p (g m d) -> p g m d", g=g, m=M)
                w_b = w3[:, :, :GL].unsqueeze(3).broadcast_to([P, g, GL, D])
                nc.vector.tensor_mul(out=wp4[:, :, :GL, :], in0=mem4, in1=w_b)
                if Hp > GL:
                    nc.vector.memset(wp4[:, :, GL:Hp, :], 0)

                fu_out = ne3[:, :, D : 2 * D]
                if Hp == 1:
                    nc.vector.tensor_copy(out=fu_out, in_=wp4[:, :, 0, :])
                elif Hp == 2:
                    nc.vector.tensor_add(
                        out=fu_out, in0=wp4[:, :, 0, :], in1=wp4[:, :, 1, :]
                    )
                else:
                    t_sb = prpool.tile([P, g * 12 * D], dt.float32, tag="tree")
                    tv = t_sb[:].rearrange("p (g m d) -> p g m d", g=g, m=12)
                    if Hp == 4:
                        nc.vector.tensor_add(
                            out=tv[:, :, 0:2, :],
                            in0=wp4[:, :, 0:2, :],
                            in1=wp4[:, :, 2:4, :],
                        )
                        nc.vector.tensor_add(
                            out=fu_out, in0=tv[:, :, 0, :], in1=tv[:, :, 1, :]
                        )
                    elif Hp == 8:
                        nc.vector.tensor_add(
                            out=tv[:, :, 0:4, :],
                            in0=wp4[:, :, 0:4, :],
                            in1=wp4[:, :, 4:8, :],
                        )
                        nc.vector.tensor_add(
                            out=tv[:, :, 4:6, :],
                            in0=tv[:, :, 0:2, :],
                            in1=tv[:, :, 2:4, :],
                        )
                        nc.vector.tensor_add(
                            out=fu_out, in0=tv[:, :, 4, :], in1=tv[:, :, 5, :]
                        )
                    else:  # Hp == 16
                        nc.vector.tensor_add(
                            out=tv[:, :, 0:8, :],
                            in0=wp4[:, :, 0:8, :],
                            in1=wp4[:, :, 8:16, :],
                        )
                        nc.vector.tensor_add(
                            out=tv[:, :, 8:12, :],
                            in0=tv[:, :, 0:4, :],
                            in1=tv[:, :, 4:8, :],
                        )
                        nc.vector.tensor_add(
                            out=tv[:, :, 0:2, :],
                            in0=tv[:, :, 8:10, :],
                            in1=tv[:, :, 10:12, :],
                        )
                        nc.vector.tensor_add(
                            out=fu_out, in0=tv[:, :, 0, :], in1=tv[:, :, 1, :]
                        )

                nc.vector.tensor_mul(
                    out=ne3[:, :, 0:D],
                    in0=ne3[:, :, D : 2 * D],
                    in1=ne3[:, :, 2 * D : 3 * D],
                )

                neT_ps = ppool.tile([3 * D, GP], dt.bfloat16, tag="neT", space="PSUM")
                for j in range(g):
                    nc.tensor.transpose(
                        out=neT_ps[:, j * P : (j + 1) * P],
                        in_=ne3[:, j, :],
                        identity=id_sb[:],
                    )
                neT_sb = wpool.tile([3 * D, GP], dt.bfloat16, tag="neTs")
                nc.scalar.activation(
                    out=neT_sb[:],
                    in_=neT_ps[:],
                    func=mybir.ActivationFunctionType.Copy,
                )

                hT_ps = ppool.tile([8, GP], dt.float32, tag="hT", space="PSUM")
                nc.tensor.matmul(
                    hT_ps[:],
                    lhsT=w1_sb[:],
                    rhs=neT_sb[:],
                    start=True,
                    stop=True,
                )
                hT_sb = wpool.tile([8, GP], dt.bfloat16, tag="hTs")
                nc.scalar.activation(
                    out=hT_sb[:],
                    in_=hT_ps[:],
                    func=mybir.ActivationFunctionType.Relu,
                    bias=b1_sb[:, :1],
                )

                yT_ps = ppool.tile([1, GP], dt.float32, tag="yT", space="PSUM")
                nc.tensor.matmul(
                    yT_ps[:],
                    lhsT=w2_sb[:],
                    rhs=hT_sb[:],
                    start=True,
                    stop=True,
                )
                y_sb = iopool.tile([1, GP], dt.float32, tag="y")
                nc.scalar.activation(
                    out=y_sb[:],
                    in_=yT_ps[:],
                    func=mybir.ActivationFunctionType.Sigmoid,
                    bias=b2_sb[:1, :1],
                )
                nc.sync.dma_start(
                    out=y_out[gi * g : (gi + 1) * g, :], in_=y_sb[:]
                )

    nc.compile()
    return nc


def _lengths_from_mask(mask_b):
    mm = np.asarray(mask_b, dtype=bool)
    pos = np.arange(1, M + 1, dtype=np.int32)
    return (mm * pos[None, :]).max(axis=1).astype(np.int32)


def prepare(item_inputs, member_ids, member_mask, n_cores=N_CORES):
    L = _lengths_from_mask(member_mask)
    order = np.argsort(-L, kind="stable")
    n = len(L)
    bc = n // n_cores
    nt = bc // P
    Ls = L[order]
    prof = [int(max(1, Ls[t * P * n_cores])) for t in range(nt)]
    return order, prof


def _wrap16(idv):
    """[n] int16 idx list -> [128, n/16] wrapped + replicated layout."""
    n = len(idv)
    w16 = idv.reshape(n // 16, 16).T
    return np.tile(w16, (8, 1))


def _make_in_maps(item_inputs, member_ids, member_mask, user_table, item_table,
                  W_bil, b_bil, W1, b1, W2, b2, order, prof, g=G):
    import ml_dtypes

    bf16 = ml_dtypes.bfloat16
    item_inputs = np.asarray(item_inputs).astype(np.int32).reshape(-1)
    member_ids = np.asarray(member_ids).astype(np.int32)
    mask_f = np.asarray(member_mask).astype(np.float32)
    user4 = np.ascontiguousarray(
        np.asarray(user_table, dtype=np.float32).astype(bf16).reshape(
            NU // 4, 4 * D
        )
    )
    item4 = np.ascontiguousarray(
        np.asarray(item_table, dtype=np.float32).astype(bf16).reshape(
            NI // 4, 4 * D
        )
    )
    w_bil_t = np.ascontiguousarray(
        np.asarray(W_bil, dtype=np.float32).T.astype(bf16)
    )
    w1_t = np.ascontiguousarray(np.asarray(W1, dtype=np.float32).T.astype(bf16))
    w2_t = np.ascontiguousarray(np.asarray(W2, dtype=np.float32).T.astype(bf16))
    b1_c = np.asarray(b1, dtype=np.float32).reshape(8, 1)
    b2_c = np.asarray(b2, dtype=np.float32).reshape(1, 1)
    bbil_c = np.full((P, 1), np.asarray(b_bil, dtype=np.float32).reshape(-1)[0],
                     dtype=np.float32)
    ident = np.eye(P, dtype=np.float32).astype(bf16)

    gls = _group_gl(prof, g)
    ngroups = len(gls)

    in_maps = []
    for c in range(N_CORES):
        rows = order[c::N_CORES]
        mi = member_ids[rows]              # [bc, M]
        ii = item_inputs[rows]             # [bc]
        idx_parts, blob_parts, iid_parts = [], [], []
        for gi in range(ngroups):
            GL = gls[gi]
            blk = mi[gi * g * P : (gi + 1) * g * P, :GL]     # [g*P, GL]
            b4 = blk.reshape(g, P, GL)
            idv = np.transpose(b4, (0, 2, 1)).reshape(-1)     # (j,m,p) order
            idx_parts.append(_wrap16((idv >> 2).astype(np.int16)))
            sub = (np.transpose(b4, (0, 2, 1)) & 3)           # [g, GL, P]
            subm = np.transpose(sub, (2, 0, 1)).reshape(P, g * GL)  # [p,(j,m)]
            ib = ii[gi * g * P : (gi + 1) * g * P].reshape(g, P)
            isub = (ib & 3).T                                  # [P, g]
            blob_parts.append(np.concatenate(
                [(subm == q).astype(np.uint8) for q in range(4)]
                + [(isub == q).astype(np.uint8) for q in range(4)], axis=1))
            iid_parts.append(((ib >> 2).astype(np.int16)))     # [g, P]
        # item idxs per quad of 4 groups: (grp_in_quad, j, p) order
        iid_quads = []
        for qk in range(ngroups // 4):
            iv = np.concatenate(
                [iid_parts[4 * qk + t].reshape(-1) for t in range(4)]
            )
            iid_quads.append(_wrap16(iv))
        im = {
            "ids16": np.concatenate(idx_parts, axis=1),
            "iid16": np.concatenate(iid_quads, axis=1),
            "mblob": np.concatenate(blob_parts, axis=1),
            "mask": np.ascontiguousarray(mask_f[rows]),
            "user4": user4,
            "item4": item4,
            "w_bil_t": w_bil_t,
            "w1_t": w1_t,
            "w2_t": w2_t,
            "b1": b1_c,
            "b2": b2_c,
            "bbil": bbil_c,
            "ident": ident,
        }
        in_maps.append(im)
    return in_maps


def _get_compiled(prof):
    key = tuple(prof)
    if key not in _COMPILED:
        _COMPILED[key] = build_kernel(BC, G, prof=list(prof))
    return _COMPILED[key]


def run_on_hw(nc, in_maps, trace=False):
    from concourse import bass_utils

    return bass_utils.run_bass_kernel_spmd(
        nc, in_maps, core_ids=list(range(N_CORES)), trace=trace
    )


def kernel(item_inputs, member_ids, member_mask, user_table, item_table,
           W_bil, b_bil, W1, b1, W2, b2):
    order, prof = prepare(item_inputs, member_ids, member_mask)
    nc = _get_compiled(prof)
    in_maps = _make_in_maps(item_inputs, member_ids, member_mask, user_table,
                            item_table, W_bil, b_bil, W1, b1, W2, b2, order, prof)
    res = run_on_hw(nc, in_maps, trace=False)
    y = np.empty(B, dtype=np.float32)
    for c in range(N_CORES):
        y[order[c::N_CORES]] = res.results[c]["y"].reshape(BC)
    return y.reshape(B, 1)


# revision 7
# speedup vs baseline: 1.1819x; 1.1819x over previous
"""Trainium2 Bass kernel for nn_BILINEAR_56169582297414 (gnn message passing).

Reference computation (per prediction pair b):
    item_e = item_table[item_inputs[b]]                    # [D]
    mem_e  = user_table[member_ids[b, :]]                  # [M, D]
    scores[m] = mem_e[m] @ W_bil @ item_e + b_bil          # bilinear
    w = scores * member_mask[b]                            # mask padded members
    fu = sum_m w[m] * mem_e[m]                             # [D]
    ne = [fu * item_e, fu, item_e]                         # [3D]
    y = sigmoid(relu(ne @ W1.T + b1) @ W2.T + b2)          # [1]

Strategy: data-parallel over 8 NeuronCores, tables replicated. The serial
resource is SWDGE gather descriptor supply (~11 ns/desc per queue pair,
~2.8 ns/desc aggregate over the 4 SWDGE queues). So:
  - rows sorted by true group length (desc) and striped across cores; each
    tile fetches only its max length (~1.9x fewer member rows);
  - member and item tables both bf16 4-packed (256B elements); gathers are
    assigned to the least-loaded of the 4 SWDGE queues (4 Q7 core pairs);
    item gathers batched 4 groups per instruction;
  - dynamic_dma_scratch_size doubled so each queue's descriptor ring holds
    ~2 gathers, reducing decode-stage await_space head-of-line blocking;
  - packed-row selects via copy_predicated only (DVE COPY opcode is ~4x
    slower) on uint32-bitcast views to halve element count;
  - the weighted member sum uses a pairwise add tree over contiguous views
    (the strided [g,d,m] TENSOR_REDUCE it replaces ran ~20x slower);
  - everything up to PSUM runs in bf16 (2x DVE and PE); PSUM accumulation
    and the score/activation nonlinearities stay fp32.
"""

import sys

sys.path.insert(0, "/opt/trn_rl_repo")

import numpy as np

B = 262144
M = 16
NU = 100000
NI = 50000
D = 32
N_CORES = 8
BC = B // N_CORES
P = 128
NT = BC // P
G = 4

_COMPILED = {}


def _group_gl(prof, g=G):
    """Per-group max member count (prof is non-increasing)."""
    return [max(prof[i * g : (i + 1) * g]) for i in range(len(prof) // g)]


def _next_pow2(x):
    h = 1
    while h < x:
        h *= 2
    return h


def _queue_plan(gls, g=G):
    """Greedy least-loaded queue for each gather, in emit order.

    Returns (member_q[gi], quad_q[qk]) lists."""
    load = [0, 0, 0, 0]
    member_q = [0] * len(gls)
    quad_q = [0] * (len(gls) // 4)
    recent = []  # last two queues in emit order - avoid back-to-back reuse

    def pick():
        cands = [i for i in range(4) if i not in recent] or list(range(4))
        q = min(cands, key=lambda i: (load[i], i))
        recent.append(q)
        if len(recent) > 2:
            recent.pop(0)
        return q

    for gi in range(len(gls)):
        if gi % 4 == 0:
            q = pick()
            quad_q[gi // 4] = q
            load[q] += 16 * 128
        q = pick()
        member_q[gi] = q
        load[q] += g * gls[gi] * 128
    return member_q, quad_q


def build_kernel(bc, g=G, prof=None):
    import concourse.bacc as bacc
    import concourse.tile as tile
    from concourse import mybir
    from concourse.library_config import mlp

    nt = bc // P
    assert nt % g == 0
    ngroups = nt // g
    assert ngroups % 4 == 0
    if prof is None:
        prof = [M] * nt
    prof = [int(max(1, min(M, x))) for x in prof]
    gls = _group_gl(prof, g)
    member_q, quad_q = _queue_plan(gls, g)
    dt = mybir.dt

    # flat col offsets for per-group idx / mask-blob tensors
    idx_cols = [g * gl * 8 for gl in gls]            # int16 cols ([128, .])
    blob_cols = [4 * g * gl + 4 * g for gl in gls]   # uint8: 4 msel + 4 isel
    idx_off = np.concatenate([[0], np.cumsum(idx_cols)]).astype(int)
    blob_off = np.concatenate([[0], np.cumsum(blob_cols)]).astype(int)

    nc = bacc.Bacc("TRN2", target_bir_lowering=False, debug=False,
                   num_swdge_queues=4, dynamic_dma_scratch_size=32768)

    ids16 = nc.dram_tensor("ids16", [P, int(idx_off[-1])], dt.int16,
                           kind="ExternalInput")
    iid16 = nc.dram_tensor("iid16", [P, (ngroups // 4) * 16 * 8], dt.int16,
                           kind="ExternalInput")
    mblob = nc.dram_tensor("mblob", [P, int(blob_off[-1])], dt.uint8,
                           kind="ExternalInput")
    mask = nc.dram_tensor("mask", [bc, M], dt.float32, kind="ExternalInput")
    user4 = nc.dram_tensor("user4", [NU // 4, 4 * D], dt.bfloat16,
                           kind="ExternalInput")
    item4 = nc.dram_tensor("item4", [NI // 4, 4 * D], dt.bfloat16,
                           kind="ExternalInput")
    w_bil_t = nc.dram_tensor("w_bil_t", [D, D], dt.bfloat16, kind="ExternalInput")
    w1_t = nc.dram_tensor("w1_t", [3 * D, 8], dt.bfloat16, kind="ExternalInput")
    w2_t = nc.dram_tensor("w2_t", [8, 1], dt.bfloat16, kind="ExternalInput")
    b1 = nc.dram_tensor("b1", [8, 1], dt.float32, kind="ExternalInput")
    b2 = nc.dram_tensor("b2", [1, 1], dt.float32, kind="ExternalInput")
    bbil = nc.dram_tensor("bbil", [P, 1], dt.float32, kind="ExternalInput")
    ident = nc.dram_tensor("ident", [P, P], dt.bfloat16, kind="ExternalInput")
    y_out = nc.dram_tensor("y", [nt, P], dt.float32, kind="ExternalOutput")

    GM = g * M
    GNE = g * 3 * D
    GP = g * P

    with tile.TileContext(nc) as tc:
        with (
            tc.tile_pool(name="const", bufs=1) as cpool,
            tc.tile_pool(name="io", bufs=6) as iopool,
            tc.tile_pool(name="work", bufs=3) as wpool,
            tc.tile_pool(name="gath", bufs=6) as gpool,
            tc.tile_pool(name="quad", bufs=2) as qpool,
            tc.tile_pool(name="prodp", bufs=2) as prpool,
            tc.tile_pool(name="psum", bufs=1, space="PSUM") as ppool,
            tc.tile_pool(name="psumv", bufs=2, space="PSUM") as ppoolv,
        ):
            with tc.tile_critical():
                nc.gpsimd.load_library(mlp)

            wt_sb = cpool.tile([D, D], dt.bfloat16, tag="wt")
            nc.sync.dma_start(out=wt_sb[:], in_=w_bil_t[:])
            w1_sb = cpool.tile([3 * D, 8], dt.bfloat16, tag="w1")
            nc.sync.dma_start(out=w1_sb[:], in_=w1_t[:])
            w2_sb = cpool.tile([8, 1], dt.bfloat16, tag="w2")
            nc.sync.dma_start(out=w2_sb[:], in_=w2_t[:])
            b1_sb = cpool.tile([8, 1], dt.float32, tag="b1")
            nc.sync.dma_start(out=b1_sb[:], in_=b1[:])
            b2_sb = cpool.tile([1, 1], dt.float32, tag="b2")
            nc.sync.dma_start(out=b2_sb[:], in_=b2[:])
            bbil_sb = cpool.tile([P, 1], dt.float32, tag="bbil")
            nc.sync.dma_start(out=bbil_sb[:], in_=bbil[:])
            id_sb = cpool.tile([P, P], dt.bfloat16, tag="ident")
            nc.sync.dma_start(out=id_sb[:], in_=ident[:])

            quad_sb = None
            for gi in range(ngroups):
                r0 = gi * g * P
                GL = gls[gi]
                Hp = _next_pow2(GL)
                nmi = g * GL * 128  # member idxs this group

                if gi % 4 == 0:
                    qk = gi // 4
                    iid_sb = iopool.tile([P, 16 * 8], dt.int16, tag="iid")
                    nc.sync.dma_start(
                        out=iid_sb[:],
                        in_=iid16[:, qk * 128 : (qk + 1) * 128],
                    )
                    quad_sb = qpool.tile([P, 16 * 4 * D], dt.bfloat16, tag="q4")
                    q4 = quad_sb[:].rearrange("p (c e) -> p c e", c=16)
                    nc.gpsimd.dma_gather(
                        out_ap=q4,
                        in_ap=item4[:],
                        idxs_ap=iid_sb[:],
                        num_idxs=16 * 128,
                        num_idxs_reg=16 * 128,
                        elem_size=4 * D,
                        single_packet=False,
                        queue_num=quad_q[qk],
                    )

                ids_sb = iopool.tile([P, idx_cols[gi]], dt.int16, tag="ids")
                nc.sync.dma_start(
                    out=ids_sb[:],
                    in_=ids16[:, int(idx_off[gi]) : int(idx_off[gi + 1])],
                )

                # Batched member gather: idx i=(j*GL+m)*128+p -> block j*GL+m,
                # partition p; each 256B element = 4 packed bf16 table rows.
                g4_sb = gpool.tile([P, g * GL * 4 * D], dt.bfloat16, tag="g4")
                g4 = g4_sb[:].rearrange("p (c e) -> p c e", c=g * GL)
                nc.gpsimd.dma_gather(
                    out_ap=g4,
                    in_ap=user4[:],
                    idxs_ap=ids_sb[:],
                    num_idxs=nmi,
                    num_idxs_reg=nmi,
                    elem_size=4 * D,
                    single_packet=False,
                    queue_num=member_q[gi],
                )

                blob_sb = iopool.tile([P, blob_cols[gi]], dt.uint8, tag="blob")
                nc.sync.dma_start(
                    out=blob_sb[:],
                    in_=mblob[:, int(blob_off[gi]) : int(blob_off[gi + 1])],
                )
                ms_sb = [
                    blob_sb[:, q * g * GL : (q + 1) * g * GL] for q in range(4)
                ]
                is_sb = [
                    blob_sb[:, 4 * g * GL + q * g : 4 * g * GL + (q + 1) * g]
                    for q in range(4)
                ]
                mask_sb = iopool.tile([P, GM], dt.float32, tag="mask")
                nc.sync.dma_start(
                    out=mask_sb[:].rearrange("p (g m) -> p g m", g=g),
                    in_=mask[r0 : r0 + g * P, :].rearrange("(g p) m -> p g m", p=P),
                )

                # 1-of-4 sub-row select on uint32 views -> mem [P,(g,GL),D] bf16
                mem_sb = wpool.tile([P, g * GL * D], dt.bfloat16, tag="mem")
                g4u = (
                    g4_sb[:]
                    .bitcast(dt.uint32)
                    .rearrange("p (c e) -> p c e", c=g * GL)
                )
                memu = (
                    mem_sb[:]
                    .bitcast(dt.uint32)
                    .rearrange("p (c e) -> p c e", c=g * GL)
                )
                HD = D // 2  # uint32 words per row
                for q in range(4):
                    nc.vector.copy_predicated(
                        out=memu,
                        mask=ms_sb[q].unsqueeze(2).broadcast_to([P, g * GL, HD]),
                        data=g4u[:, :, q * HD : (q + 1) * HD],
                    )

                ne_sb = wpool.tile([P, GNE], dt.bfloat16, tag="ne")
                ne3 = ne_sb[:].rearrange("p (g c) -> p g c", g=g)
                neu = (
                    ne_sb[:]
                    .bitcast(dt.uint32)
                    .rearrange("p (g c) -> p g c", g=g)
                )
                q4u = (
                    quad_sb[:]
                    .bitcast(dt.uint32)
                    .rearrange("p (c e) -> p c e", c=16)
                )
                qbase = (gi % 4) * g
                for q in range(4):
                    nc.vector.copy_predicated(
                        out=neu[:, :, D : D + HD],
                        mask=is_sb[q].unsqueeze(2).broadcast_to([P, g, HD]),
                        data=q4u[:, qbase : qbase + g, q * HD : (q + 1) * HD],
                    )

                itemT_ps = ppool.tile([D, GP], dt.bfloat16, tag="itemT", space="PSUM")
                for j in range(g):
                    nc.tensor.transpose(
                        out=itemT_ps[:, j * P : (j + 1) * P],
                        in_=ne3[:, j, 2 * D : 3 * D],
                        identity=id_sb[:],
                    )
                itemT_sb = wpool.tile([D, GP], dt.bfloat16, tag="itemT")
                nc.scalar.activation(
                    out=itemT_sb[:],
                    in_=itemT_ps[:],
                    func=mybir.ActivationFunctionType.Copy,
                )

                v_ps = ppoolv.tile([P, g * D], dt.float32, tag="v", space="PSUM")
                for j in range(g):
                    nc.tensor.matmul(
                        v_ps[:, j * D : (j + 1) * D],
                        lhsT=itemT_sb[:, j * P : (j + 1) * P],
                        rhs=wt_sb[:],
                        start=True,
                        stop=True,
                    )
                v16_sb = wpool.tile([P, g * D], dt.bfloat16, tag="v16")
                nc.scalar.activation(
                    out=v16_sb[:],
                    in_=v_ps[:],
                    func=mybir.ActivationFunctionType.Copy,
                )

                mem4 = mem_sb[:].rearrange("p (g m d) -> p g m d", g=g, m=GL)
                v_b = (
                    v16_sb[:]
                    .rearrange("p (g d) -> p g d", g=g)
                    .unsqueeze(2)
                    .broadcast_to([P, g, GL, D])
                )
                prod_sb = prpool.tile([P, GM * D], dt.bfloat16, tag="prod")
                prod4 = prod_sb[:].rearrange("p (g m d) -> p g m d", g=g, m=M)[
                    :, :, :GL, :
                ]
                nc.vector.tensor_mul(out=prod4, in0=mem4, in1=v_b)

                scores_sb = wpool.tile([P, GM], dt.float32, tag="scores")
                sc3 = scores_sb[:].rearrange("p (g m) -> p g m", g=g)
                nc.vector.reduce_sum(
                    out=sc3[:, :, :GL], in_=prod4, axis=mybir.AxisListType.X
                )

                w16_sb = wpool.tile([P, GM], dt.bfloat16, tag="w16")
                w3 = w16_sb[:].rearrange("p (g m) -> p g m", g=g)
                m3 = mask_sb[:].rearrange("p (g m) -> p g m", g=g)
                nc.vector.scalar_tensor_tensor(
                    out=w3[:, :, :GL],
                    in0=sc3[:, :, :GL],
                    scalar=bbil_sb[:, :1],
                    in1=m3[:, :, :GL],
                    op0=mybir.AluOpType.add,
                    op1=mybir.AluOpType.mult,
                )

                # weighted members; zero-pad to next pow2 for the add tree
                wp_sb = prpool.tile([P, GM * D], dt.bfloat16, tag="wprod")
                wp4 = wp_sb[:].rearrange("p (g m d) -> p g m d", g=g, m=M)
                w_b = w3[:, :, :GL].unsqueeze(3).broadcast_to([P, g, GL, D])
                nc.vector.tensor_mul(out=wp4[:, :, :GL, :], in0=mem4, in1=w_b)
                if Hp > GL:
                    nc.vector.memset(wp4[:, :, GL:Hp, :], 0)

                fu_out = ne3[:, :, D : 2 * D]
                if Hp == 1:
                    nc.vector.tensor_copy(out=fu_out, in_=wp4[:, :, 0, :])
                elif Hp == 2:
                    nc.vector.tensor_add(
                        out=fu_out, in0=wp4[:, :, 0, :], in1=wp4[:, :, 1, :]
                    )
                else:
                    t_sb = prpool.tile([P, g * 12 * D], dt.float32, tag="tree")
                    tv = t_sb[:].rearrange("p (g m d) -> p g m d", g=g, m=12)
                    if Hp == 4:
                        nc.vector.tensor_add(
                            out=tv[:, :, 0:2, :],
                            in0=wp4[:, :, 0:2, :],
                            in1=wp4[:, :, 2:4, :],
                        )
                        nc.vector.tensor_add(
                            out=fu_out, in0=tv[:, :, 0, :], in1=tv[:, :, 1, :]
                        )
                    elif Hp == 8:
                        nc.vector.tensor_add(
                            out=tv[:, :, 0:4, :],
                            in0=wp4[:, :, 0:4, :],
                            in1=wp4[:, :, 4:8, :],
                        )
                        nc.vector.tensor_add(
                            out=tv[:, :, 4:6, :],
                            in0=tv[:, :, 0:2, :],
                            in1=tv[:, :, 2:4, :],
                        )
                        nc.vector.tensor_add(
                            out=fu_out, in0=tv[:, :, 4, :], in1=tv[:, :, 5, :]
                        )
                    else:  # Hp == 16
                        nc.vector.tensor_add(
                            out=tv[:, :, 0:8, :],
                            in0=wp4[:, :, 0:8, :],
                            in1=wp4[:, :, 8:16, :],
                        )
                        nc.vector.tensor_add(
                            out=tv[:, :, 8:12, :],
                            in0=tv[:, :, 0:4, :],
                            in1=tv[:, :, 4:8, :],
                        )
                        nc.vector.tensor_add(
                            out=tv[:, :, 0:2, :],
                            in0=tv[:, :, 8:10, :],
                            in1=tv[:, :, 10:12, :],
                        )
                        nc.vector.tensor_add(
                            out=fu_out, in0=tv[:, :, 0, :], in1=tv[:, :, 1, :]
                        )

                nc.vector.tensor_mul(
                    out=ne3[:, :, 0:D],
                    in0=ne3[:, :, D : 2 * D],
                    in1=ne3[:, :, 2 * D : 3 * D],
                )

                neT_ps = ppool.tile([3 * D, GP], dt.bfloat16, tag="neT", space="PSUM")
                for j in range(g):
                    nc.tensor.transpose(
                        out=neT_ps[:, j * P : (j + 1) * P],
                        in_=ne3[:, j, :],
                        identity=id_sb[:],
                    )
                neT_sb = wpool.tile([3 * D, GP], dt.bfloat16, tag="neTs")
                nc.scalar.activation(
                    out=neT_sb[:],
                    in_=neT_ps[:],
                    func=mybir.ActivationFunctionType.Copy,
                )

                hT_ps = ppool.tile([8, GP], dt.float32, tag="hT", space="PSUM")
                nc.tensor.matmul(
                    hT_ps[:],
                    lhsT=w1_sb[:],
                    rhs=neT_sb[:],
                    start=True,
                    stop=True,
                )
                hT_sb = wpool.tile([8, GP], dt.bfloat16, tag="hTs")
                nc.scalar.activation(
                    out=hT_sb[:],
                    in_=hT_ps[:],
                    func=mybir.ActivationFunctionType.Relu,
                    bias=b1_sb[:, :1],
                )

                yT_ps = ppool.tile([1, GP], dt.float32, tag="yT", space="PSUM")
                nc.tensor.matmul(
                    yT_ps[:],
                    lhsT=w2_sb[:],
                    rhs=hT_sb[:],
                    start=True,
                    stop=True,
                )
                y_sb = iopool.tile([1, GP], dt.float32, tag="y")
                nc.scalar.activation(
                    out=y_sb[:],
                    in_=yT_ps[:],
                    func=mybir.ActivationFunctionType.Sigmoid,
                    bias=b2_sb[:1, :1],
                )
                nc.sync.dma_start(
                    out=y_out[gi * g : (gi + 1) * g, :], in_=y_sb[:]
                )

    nc.compile()
    return nc


def _lengths_from_mask(mask_b):
    mm = np.asarray(mask_b, dtype=bool)
    pos = np.arange(1, M + 1, dtype=np.int32)
    return (mm * pos[None, :]).max(axis=1).astype(np.int32)


def prepare(item_inputs, member_ids, member_mask, n_cores=N_CORES):
    L = _lengths_from_mask(member_mask)
    order = np.argsort(-L, kind="stable")
    n = len(L)
    bc = n // n_cores
    nt = bc // P
    Ls = L[order]
    prof = [int(max(1, Ls[t * P * n_cores])) for t in range(nt)]
    return order, prof


def _wrap16(idv):
    """[n] int16 idx list -> [128, n/16] wrapped + replicated layout."""
    n = len(idv)
    w16 = idv.reshape(n // 16, 16).T
    return np.tile(w16, (8, 1))


def _make_in_maps(item_inputs, member_ids, member_mask, user_table, item_table,
                  W_bil, b_bil, W1, b1, W2, b2, order, prof, g=G):
    import ml_dtypes

    bf16 = ml_dtypes.bfloat16
    item_inputs = np.asarray(item_inputs).astype(np.int32).reshape(-1)
    member_ids = np.asarray(member_ids).astype(np.int32)
    mask_f = np.asarray(member_mask).astype(np.float32)
    user4 = np.ascontiguousarray(
        np.asarray(user_table, dtype=np.float32).astype(bf16).reshape(
            NU // 4, 4 * D
        )
    )
    item4 = np.ascontiguousarray(
        np.asarray(item_table, dtype=np.float32).astype(bf16).reshape(
            NI // 4, 4 * D
        )
    )
    w_bil_t = np.ascontiguousarray(
        np.asarray(W_bil, dtype=np.float32).T.astype(bf16)
    )
    w1_t = np.ascontiguousarray(np.asarray(W1, dtype=np.float32).T.astype(bf16))
    w2_t = np.ascontiguousarray(np.asarray(W2, dtype=np.float32).T.astype(bf16))
    b1_c = np.asarray(b1, dtype=np.float32).reshape(8, 1)
    b2_c = np.asarray(b2, dtype=np.float32).reshape(1, 1)
    bbil_c = np.full((P, 1), np.asarray(b_bil, dtype=np.float32).reshape(-1)[0],
                     dtype=np.float32)
    ident = np.eye(P, dtype=np.float32).astype(bf16)

    gls = _group_gl(prof, g)
    ngroups = len(gls)

    in_maps = []
    for c in range(N_CORES):
        rows = order[c::N_CORES]
        mi = member_ids[rows]              # [bc, M]
        ii = item_inputs[rows]             # [bc]
        idx_parts, blob_parts, iid_parts = [], [], []
        for gi in range(ngroups):
            GL = gls[gi]
            blk = mi[gi * g * P : (gi + 1) * g * P, :GL]     # [g*P, GL]
            b4 = blk.reshape(g, P, GL)
            idv = np.transpose(b4, (0, 2, 1)).reshape(-1)     # (j,m,p) order
            idx_parts.append(_wrap16((idv >> 2).astype(np.int16)))
            sub = (np.transpose(b4, (0, 2, 1)) & 3)           # [g, GL, P]
            subm = np.transpose(sub, (2, 0, 1)).reshape(P, g * GL)  # [p,(j,m)]
            ib = ii[gi * g * P : (gi + 1) * g * P].reshape(g, P)
            isub = (ib & 3).T                                  # [P, g]
            blob_parts.append(np.concatenate(
                [(subm == q).astype(np.uint8) for q in range(4)]
                + [(isub == q).astype(np.uint8) for q in range(4)], axis=1))
            iid_parts.append(((ib >> 2).astype(np.int16)))     # [g, P]
        # item idxs per quad of 4 groups: (grp_in_quad, j, p) order
        iid_quads = []
        for qk in range(ngroups // 4):
            iv = np.concatenate(
                [iid_parts[4 * qk + t].reshape(-1) for t in range(4)]
            )
            iid_quads.append(_wrap16(iv))
        im = {
            "ids16": np.concatenate(idx_parts, axis=1),
            "iid16": np.concatenate(iid_quads, axis=1),
            "mblob": np.concatenate(blob_parts, axis=1),
            "mask": np.ascontiguousarray(mask_f[rows]),
            "user4": user4,
            "item4": item4,
            "w_bil_t": w_bil_t,
            "w1_t": w1_t,
            "w2_t": w2_t,
            "b1": b1_c,
            "b2": b2_c,
            "bbil": bbil_c,
            "ident": ident,
        }
        in_maps.append(im)
    return in_maps


def _get_compiled(prof):
    key = tuple(prof)
    if key not in _COMPILED:
        _COMPILED[key] = build_kernel(BC, G, prof=list(prof))
    return _COMPILED[key]


def run_on_hw(nc, in_maps, trace=False):
    from concourse import bass_utils

    return bass_utils.run_bass_kernel_spmd(
        nc, in_maps, core_ids=list(range(N_CORES)), trace=trace
    )


def kernel(item_inputs, member_ids, member_mask, user_table, item_table,
           W_bil, b_bil, W1, b1, W2, b2):
    order, prof = prepare(item_inputs, member_ids, member_mask)
    nc = _get_compiled(prof)
    in_maps = _make_in_maps(item_inputs, member_ids, member_mask, user_table,
                            item_table, W_bil, b_bil, W1, b1, W2, b2, order, prof)
    res = run_on_hw(nc, in_maps, trace=False)
    y = np.empty(B, dtype=np.float32)
    for c in range(N_CORES):
        y[order[c::N_CORES]] = res.results[c]["y"].reshape(BC)
    return y.reshape(B, 1)
